# revision 14
# baseline (speedup 1.0000x reference)
"""Trainium2 Bass kernel for nn_DigitConvolutionalModel (3x3 conv + 3-layer MLP).

Math: out = relu(relu(conv3x3(x) @ W1 + b1) @ W2 + b2) @ W3 + b3.

The 3x3 valid conv is linear, so on host we fold it into the first FC:
  h1 = relu(x @ W1eff + b1)  with  W1eff = A @ W1 : [784, 256].
K = 784 is tiled as SEVEN 112-row k-tiles (784 = 7*112): uniform pieces,
no 16-row tail special case (PE matmul cost is N-bound, independent of K
below 128).  Each piece carries 113 partition rows: rows 0-111 hold
W1eff/x data, row 112 holds b1|ones on the k6 piece (the bias rides the
matmul) and zeros elsewhere.  b2/b3 ride as fp16 columns of the wa
tensor (one DVE copy stages them to fp32).

Sharding: pure data parallel over the batch across 8 cores (2048 rows each).
Feature-major 3-layer MLP (activations transposed; zero on-device transposes):
  h1T = relu(W1eff.T @ xT [+b1 via ones-row])   [256, 2048]
  h2T = relu(W2.T   @ h1T + b2)                 [256, 2048]
  oT  =      W3.T   @ h2T + b3                  [10, 2048]
Matmuls in fp16 (full-rate PE) with fp32 PSUM accumulation.

Schedule rationale (from trace analysis):
 - The PE HAM clock gate holds the PE at 1.2GHz until ~3.4us of SUSTAINED
   busy-ness; gaps in that burst delay the 2.4GHz un-throttle, and any
   >3.4us idle re-throttles.  The warm-up burst runs gap-free from
   engine-alive (~1.1us) into the first data arrival.
 - HWDGE facts (measured): first bytes land ~2.4-2.9us; each dma_start
   costs its sequencer ~0.65us of serial descgen; each ring sustains only
   ~4 in-flight transfers (the 5th dma_start blocks until a completion);
   a transfer's semaphore fires ~0.5us after its last byte on a quiet
   ring but ~2us under load.  So chunk-0's pieces go 4-per-ring with NO
   other traffic queued behind them, and prefetches enter the rings only
   as the windows free up (which the depth limit enforces naturally).
 - Transfers with few partition rows drain on few SDMA engines (the
   partition->port swizzle) -- the uniform 113-row pieces avoid that trap.
 - Chunk sizes ramp [256,512,512,512,256]: chunk-0 is cheap to fill,
   steady-state chunks are PE-bound, and the small last chunk shortens
   the serial relu->L2->relu->L3->store dependency tail (its L1 runs
   m-outer so the m0 relu overlaps the m1 pass).
 - Output is stored as fp16 (cast up on host; ~1e-4 extra rel err); all
   stores ride SWDGE (gpsimd), never touching the load rings.
"""

import numpy as np

import concourse.bacc as bacc
import concourse.bass as bass
import concourse.mybir as mybir
import concourse.tile as tile
from concourse.bass_utils import run_bass_kernel_spmd

N_CORES = 8
B = 16384
B_LOC = B // N_CORES  # 2048 batch rows per core
CS = [256, 512, 512, 512, 256]  # ramped chunk sizes
CO = [0, 256, 768, 1280, 1792]  # chunk offsets
NCHUNKS = len(CS)
KIN = 784  # folded input features (28*28)
NKT = 7  # k-tiles of 112 rows each (784 = 7*112)
KR = 112  # partition rows per piece -- a multiple of 16 (HWDGE descgen
# bulk path and SDMA striping both require it; 113 was catastrophic)
H = 256
NOUT = 10
NWARM = 34  # warm-up matmuls bridging engine-alive (~1.7us) -> first piece
# semaphore (~5.5us), gap-free, so HAM un-throttles before real work

C0W = 256 + CS[0]  # 512 cols per chunk-0 [w_k | x0_k] piece
WA_W3 = 2 * H  # col offset of w3 block in wa
WA_B1 = WA_W3 + 2 * NOUT  # col offset of b1 (2 cols)
WA_B2 = WA_B1 + 2  # col offset of b2 (2 cols)
WA_B3 = WA_B2 + 2  # col offset of b3 (1 col)
WA_COLS = WA_B3 + 1

F32 = mybir.dt.float32
F16 = mybir.dt.float16
AF = mybir.ActivationFunctionType
ALU = mybir.AluOpType


def build_nc() -> bass.Bass:
    nc = bacc.Bacc(
        "TRN2", target_bir_lowering=False, debug=False, num_devices=N_CORES
    )
    # Host-packed inputs (exact SBUF destination layouts; xT = x_shard.T):
    #   c0p[p][k*512+c]: c<256 -> W1eff[k*112+p, c]; c>=256 -> xT[k*112+p, c-256]
    #                    row 112: k==6 -> b1|ones, else zeros
    #   wa[p][c]: c<512 k-major W2; then k-major W3 (20); then b2 (2), b3 (1)
    #   xc[i][p][k*512+n] -> xT[k*112+p, CO[i+1]+n]  (chunks 1-3; k6 row 112=1)
    #   x4[p][k*256+n]   -> xT[k*112+p, 1792+n]      (chunk 4)
    c0p = nc.dram_tensor("c0p", [KR, NKT * C0W], F16, kind="ExternalInput")
    wa = nc.dram_tensor("wa", [128, WA_COLS], F16, kind="ExternalInput")
    xc = nc.dram_tensor("xc", [3, KR, NKT * 512], F16, kind="ExternalInput")
    x4 = nc.dram_tensor("x4", [KR, NKT * 256], F16, kind="ExternalInput")
    outT = nc.dram_tensor("outT", [NOUT, B_LOC], F16, kind="ExternalOutput")

    with tile.TileContext(nc) as tc:
        with (
            tc.tile_pool(name="wgt", bufs=1) as wp,
            tc.tile_pool(name="xin", bufs=3) as xp,
            tc.tile_pool(name="act", bufs=3) as hp,
            tc.tile_pool(name="osb", bufs=2) as op,
            tc.tile_pool(name="ps1", bufs=2, space="PSUM") as pp1,
            tc.tile_pool(name="ps2", bufs=2, space="PSUM") as pp2,
        ):
            # PE warm-up: small matmuls on a zeroed scratch tile, no DMA deps.
            warm = wp.tile([128, 128], F16, name="warm")
            nc.vector.memset(warm[:], 0.0)
            psw = pp1.tile([128, 512], F32, name="psw", tag="ps1_0")
            for _ in range(NWARM):
                nc.tensor.matmul(
                    psw[:, 0:128], warm[:], warm[:], start=True, stop=True
                )

            # ---- chunk-0 self-contained [w|x] pieces, alternating across the
            # two HWDGE rings in consumption order; exactly 4 per ring so the
            # in-flight window stays clear of prefetch traffic ----
            c0t = [wp.tile([KR, C0W], F16, name=f"c0k{k}") for k in range(NKT)]
            wat = wp.tile([128, WA_COLS], F16, name="wat")

            for k in range(NKT):
                eng = nc.sync if k % 2 == 0 else nc.scalar
                eng.dma_start(out=c0t[k][:], in_=c0p[:, k * C0W : (k + 1) * C0W])
            nc.scalar.dma_start(out=wat[:], in_=wa[:, :])

            # later-chunk x prefetches: A half = k0-2, B half = k3-6,
            # halves alternated across rings to balance bytes
            xa_t = {}
            xb_t = {}
            for ci in range(1, 4):
                ea = nc.sync if ci % 2 == 1 else nc.scalar
                eb = nc.scalar if ci % 2 == 1 else nc.sync
                xa_t[ci] = xp.tile([KR, 3 * 512], F16, name=f"xa{ci}", tag="xa")
                ea.dma_start(out=xa_t[ci][:], in_=xc[ci - 1, :, 0 : 3 * 512])
                xb_t[ci] = xp.tile([KR, 4 * 512], F16, name=f"xb{ci}", tag="xb")
                eb.dma_start(
                    out=xb_t[ci][:], in_=xc[ci - 1, :, 3 * 512 : 7 * 512]
                )
            xa_t[4] = xp.tile([KR, 3 * 256], F16, name="xa4")
            nc.scalar.dma_start(out=xa_t[4][:], in_=x4[:, 0 : 3 * 256])
            xb_t[4] = xp.tile([KR, 4 * 256], F16, name="xb4")
            nc.sync.dma_start(out=xb_t[4][:], in_=x4[:, 3 * 256 : 7 * 256])

            def w1_piece(k, m):
                return c0t[k][0:KR, m * 128 : (m + 1) * 128]

            def x_piece(ci, k):
                cs = CS[ci]
                if ci == 0:
                    return c0t[k][0:KR, 256 : 256 + cs]
                xt = xa_t[ci] if k < 3 else xb_t[ci]
                kk = k if k < 3 else k - 3
                return xt[0:KR, kk * cs : (kk + 1) * cs]

            # fp32 staging of b2/b3 (tensor_scalar needs fp32 scalar APs);
            # one DVE copy, far off the critical path.
            bf = wp.tile([128, 5], F32, name="bf")
            nc.vector.tensor_copy(bf[:], wat[:, WA_B1 : WA_B3 + 1])
            b1m = [bf[:, 0:1], bf[:, 1:2]]
            b2m = [bf[:, 2:3], bf[:, 3:4]]
            b3v = bf[0:NOUT, 4:5]

            # ---- batch-chunk pipeline ----
            for ci in range(NCHUNKS):
                cs = CS[ci]
                n0 = CO[ci]
                last = ci == NCHUNKS - 1

                # layer 1.  k-outer/m-inner so each arriving piece feeds both
                # m matmuls at once; the LAST chunk runs m-outer so ps1_0
                # completes early and its relu overlaps the m1 pass.
                ps1f = [
                    pp1.tile([128, 512], F32, name="ps1", tag=f"ps1_{m}")
                    for m in range(2)
                ]
                ps1 = [p[:, 0:cs] for p in ps1f]
                if not last:
                    for k in range(NKT):
                        xv = x_piece(ci, k)
                        for m in range(2):
                            nc.tensor.matmul(
                                ps1[m],
                                w1_piece(k, m),
                                xv,
                                start=(k == 0),
                                stop=(k == NKT - 1),
                            )
                else:
                    for m in range(2):
                        for k in range(NKT):
                            nc.tensor.matmul(
                                ps1[m],
                                w1_piece(k, m),
                                x_piece(ci, k),
                                start=(k == 0),
                                stop=(k == NKT - 1),
                            )

                h1 = []
                for m in range(2):
                    hf = hp.tile([128, 512], F16, name="h1", tag=f"h1_{m}")
                    h = hf[:, 0:cs]
                    if m == 0:
                        nc.scalar.activation(h, ps1[m], AF.Relu, bias=b1m[m])
                    else:
                        nc.vector.tensor_scalar(
                            h, ps1[m], b1m[m], 0.0, ALU.add, ALU.max
                        )
                    h1.append(h)

                # layer 2: h2T = relu(W2.T @ h1T + b2)
                h2 = []
                for m in range(2):
                    psf = pp2.tile([128, 512], F32, name="ps2", tag=f"ps2_{m}")
                    ps = psf[:, 0:cs]
                    for k in range(2):
                        nc.tensor.matmul(
                            ps,
                            wat[:, k * H + m * 128 : k * H + (m + 1) * 128],
                            h1[k],
                            start=(k == 0),
                            stop=(k == 1),
                        )
                    hf = hp.tile([128, 512], F16, name="h2", tag=f"h2_{m}")
                    h = hf[:, 0:cs]
                    if m == 0:
                        nc.scalar.activation(h, ps, AF.Relu, bias=b2m[m])
                    else:
                        nc.vector.tensor_scalar(
                            h, ps, b2m[m], 0.0, ALU.add, ALU.max
                        )
                    h2.append(h)

                # layer 3: oT = W3.T @ h2T + b3 (shares ps2_1 bank slots)
                ps3f = pp2.tile([128, 512], F32, name="ps3", tag="ps2_1")
                ps = ps3f[0:NOUT, 0:cs]
                for k in range(2):
                    nc.tensor.matmul(
                        ps,
                        wat[:, WA_W3 + k * NOUT : WA_W3 + (k + 1) * NOUT],
                        h2[k],
                        start=(k == 0),
                        stop=(k == 1),
                    )
                obf = op.tile([NOUT, 512], F16, name="ob", tag="ob")
                ob = obf[:, 0:cs]
                nc.vector.tensor_scalar(ob, ps, b3v, None, ALU.add)
                nc.gpsimd.dma_start(out=outT[:, n0 : n0 + cs], in_=ob)

    nc.compile()
    return nc


def _fold_conv_into_w1(conv_w: np.ndarray, W1: np.ndarray) -> np.ndarray:
    """W1eff[784, 256] such that x @ W1eff == conv_flat(x, conv_w) @ W1."""
    W1v = W1.astype(np.float64).reshape(26, 26, W1.shape[1])
    cw = conv_w.astype(np.float64)
    acc = np.zeros((28, 28, W1.shape[1]), np.float64)
    for di in range(3):
        for dj in range(3):
            acc[di : di + 26, dj : dj + 26, :] += cw[di, dj] * W1v
    return acc.reshape(KIN, W1.shape[1]).astype(np.float32)


def _pack_kmajor(w: np.ndarray, kpad: int) -> np.ndarray:
    """[K, C] -> [128, (K/128)*C] with row-block k at column block k."""
    k, c = w.shape
    wp = np.zeros((kpad, c), w.dtype)
    wp[:k] = w
    return np.ascontiguousarray(
        wp.reshape(kpad // 128, 128, c).transpose(1, 0, 2).reshape(128, -1)
    )


def _run(inputs: dict, trace: bool = False, tmpdir: str | None = None):
    x = np.asarray(inputs["x"], dtype=np.float32)
    w1e = _fold_conv_into_w1(
        np.asarray(inputs["conv_w"]), np.asarray(inputs["W1"])
    ).astype(np.float16)
    w2P = _pack_kmajor(np.asarray(inputs["W2"], np.float16), H)
    w3P = _pack_kmajor(np.asarray(inputs["W3"], np.float16), H)
    wa = np.zeros((128, WA_COLS), np.float16)
    wa[:, : 2 * H] = w2P
    wa[:, WA_W3 : WA_W3 + 2 * NOUT] = w3P
    wa[:, WA_B1 : WA_B1 + 2] = (
        np.asarray(inputs["b1"], np.float16).reshape(2, 128).T
    )
    wa[:, WA_B2 : WA_B2 + 2] = (
        np.asarray(inputs["b2"], np.float16).reshape(2, 128).T
    )
    wa[:NOUT, WA_B3] = np.asarray(inputs["b3"], np.float16)

    nc = build_nc()
    in_maps = []
    for c in range(N_CORES):
        xs = x[c * B_LOC : (c + 1) * B_LOC].astype(np.float16)  # [2048, 784]
        xsT = np.ascontiguousarray(xs.T)  # [784, 2048]
        c0pc = np.empty((KR, NKT * C0W), np.float16)
        for k in range(NKT):
            c0pc[:, k * C0W : k * C0W + 256] = w1e[k * KR : (k + 1) * KR]
            c0pc[:, k * C0W + 256 : (k + 1) * C0W] = xsT[
                k * KR : (k + 1) * KR, : CS[0]
            ]
        xcc = np.empty((3, KR, NKT * 512), np.float16)
        for ci in range(1, 4):
            for k in range(NKT):
                xcc[ci - 1, :, k * 512 : (k + 1) * 512] = xsT[
                    k * KR : (k + 1) * KR, CO[ci] : CO[ci] + 512
                ]
        x4c = np.empty((KR, NKT * 256), np.float16)
        for k in range(NKT):
            x4c[:, k * 256 : (k + 1) * 256] = xsT[
                k * KR : (k + 1) * KR, CO[4] : CO[4] + 256
            ]
        in_maps.append({"c0p": c0pc, "wa": wa, "xc": xcc, "x4": x4c})

    try:
        res = run_bass_kernel_spmd(
            nc, in_maps, list(range(N_CORES)), trace=trace, tmpdir=tmpdir
        )
    except Exception:
        # A prior session can leave a NeuronCore wedged
        # (NRT_EXEC_UNIT_UNRECOVERABLE); a retry with core reset recovers.
        import os

        os.environ.setdefault("NEURON_RT_RESET_CORES", "1")
        res = run_bass_kernel_spmd(
            nc, in_maps, list(range(N_CORES)), trace=trace, tmpdir=tmpdir
        )
    out = np.concatenate(
        [r["outT"].astype(np.float32).T for r in res.results], axis=0
    )
    return np.ascontiguousarray(out), res


def kernel(**inputs) -> np.ndarray:
    out, _ = _run(inputs, trace=False)
    return out
